# revision 1
# baseline (speedup 1.0000x reference)
"""Trainium2 Bass kernel for nn_Attention_41704132444382.

Masked-linear QKV projection + 16-head attention + masked-linear output
projection, tensor-parallel over heads across 8 NeuronCores (2 heads/core).

Layout strategy (all chosen to avoid on-device transposes of activations):
  - x is passed host-transposed as xT [1024, 4096] (k on partitions).
  - Q^T, K^T computed as [128 (2 heads x 64 d), 4096 t]  (d on partitions).
  - V^T computed the same way, then PE-transposed to V [t, dv] tiles with a
    ones column appended -> PV matmul yields attnout^T AND the softmax
    denominator (colsum) in one accumulation chain (M=65).
  - Scores computed as S^T [j keys on partitions, i queries free]; exp runs
    on ScalarE directly from PSUM with the 1/32 scale folded in (no max
    subtraction needed: |scores/32| <= ~7 so exp is safe in fp32).
  - Normalization: colsum rows are PE-transpose-gathered onto partitions,
    reciprocal on DVE, PE K=1-broadcast back to [64, i], fused into the
    PSUM->SBUF evacuation of attnout^T via tensor_tensor mult.
  - Output projection consumes attnT (dv on partitions) as lhsT directly;
    per-core partial outputs are summed on host; bias applied on host.

Matmuls use float32r (fp32 storage, 1 cyc/row on PE when N>=256 vs fp32's 4).
Set BASS_ATTN_F32R=0 to fall back to plain fp32 matmuls.
"""

import os
import sys

import numpy as np

sys.path.insert(0, "/opt/trn_rl_repo")

import concourse.bass as bass
import concourse.mybir as mybir
from concourse import bacc
from concourse.masks import make_identity
from concourse.tile import TileContext

DIM = 1024
HEADS = 16
B = 2
N = 2048
T = B * N  # 4096 flattened tokens
NCORES = 8
HPC = HEADS // NCORES  # 2 heads per core
DV = HPC * 64  # 128 head-dims per core
SCALE = DIM ** (-0.5)  # 1/32

F32 = mybir.dt.float32
F32R = mybir.dt.float32r

# matmul-operand dtype: "f32r" (default), "f32", or "bf16"
MM_DTYPE_NAME = os.environ.get("BASS_ATTN_MM_DTYPE", "f32r")
DT_MM = {"f32r": mybir.dt.float32r, "f32": F32, "bf16": mybir.dt.bfloat16}[MM_DTYPE_NAME]


def build_nc():
    nc = bacc.Bacc("TRN2", target_bir_lowering=True)
    xT_d = nc.declare_dram_parameter("xT", [DIM, T], F32, isOutput=False)
    wqkvT_d = nc.declare_dram_parameter("wqkvT", [DIM, 384], F32, isOutput=False)
    mqkvT_d = nc.declare_dram_parameter("mqkvT", [DIM, 384], F32, isOutput=False)
    woT_d = nc.declare_dram_parameter("woT", [DV, DIM], F32, isOutput=False)
    moT_d = nc.declare_dram_parameter("moT", [DV, DIM], F32, isOutput=False)
    out_d = nc.declare_dram_parameter("out", [T, DIM], F32, isOutput=True)

    gt = mybir.AluOpType.is_gt
    mult = mybir.AluOpType.mult
    Exp = mybir.ActivationFunctionType.Exp

    with TileContext(nc) as tc:
        with tc.tile_pool(name="persist", bufs=1) as pp:
            wqkv_g = pp.tile([128, 8 * 384], DT_MM)  # [k-part, (kt, o)]
            wo_g = pp.tile([128, 1024], DT_MM)
            qT = pp.tile([128, 4096], DT_MM)
            kTt = pp.tile([128, 4096], DT_MM)
            v1 = pp.tile([128, 32 * 65], DT_MM)  # [t-part, (jt, dv|1)] head 1
            v2 = pp.tile([128, 32 * 65], DT_MM)
            attnT = [pp.tile([128, 2048], DT_MM, name=f"attnT{bb}") for bb in range(B)]
            ident = pp.tile([128, 128], F32)
            ones1 = pp.tile([1, 64], DT_MM)

            make_identity(nc, ident[:])
            # memset can't emit float32r; memset f32 then cast-copy
            ones_f = pp.tile([128, 64], F32)
            nc.vector.memset(ones_f[:], 1.0)
            nc.vector.tensor_copy(ones1[:], ones_f[0:1, :])
            ones32 = pp.tile([128, 32], DT_MM)
            nc.vector.tensor_copy(ones32[:], ones_f[:, 0:32])
            # ones column at slot 64 of each 65-wide block of v1/v2 (strided write);
            # V evacuations only write cols 0..63 of each block.
            for vv in (v1, v2):
                nc.vector.tensor_copy(
                    vv[:].rearrange("p (j c) -> p j c", c=65)[:, :, 64:65],
                    ones32[:].rearrange("p (j c) -> p j c", c=1),
                )

            # ---------- Phase 0: load + gate weights ----------
            with tc.tile_pool(name="wload", bufs=2) as wl:
                wraw = wl.tile([128, 8 * 384], F32, tag="w")
                mraw = wl.tile([128, 8 * 384], F32, tag="w")
                g = wl.tile([128, 8 * 384], F32, tag="g")
                nc.sync.dma_start(
                    wraw[:].rearrange("p (kt o) -> p kt o", kt=8),
                    wqkvT_d[:].rearrange("(kt p) o -> p kt o", p=128),
                )
                nc.sync.dma_start(
                    mraw[:].rearrange("p (kt o) -> p kt o", kt=8),
                    mqkvT_d[:].rearrange("(kt p) o -> p kt o", p=128),
                )
                nc.vector.tensor_scalar(g[:], mraw[:], 0.0, None, gt)
                nc.vector.tensor_tensor(wqkv_g[:], wraw[:], g[:], mult)

                wor = wl.tile([128, 1024], F32, tag="wo")
                mor = wl.tile([128, 1024], F32, tag="wo")
                go = wl.tile([128, 1024], F32, tag="go")
                nc.sync.dma_start(wor[:], woT_d[:])
                nc.sync.dma_start(mor[:], moT_d[:])
                nc.vector.tensor_scalar(go[:], mor[:], 0.0, None, gt)
                nc.vector.tensor_tensor(wo_g[:], wor[:], go[:], mult)

            # ---------- Phase 1: QKV projection (+ V^T transpose) ----------
            vT = pp.tile([128, 4096], F32)
            with (
                tc.tile_pool(name="xq", bufs=16) as xp,
                tc.tile_pool(name="qk_ps", bufs=4, space="PSUM") as qkps,
            ):
                for q in range(4):  # t-quarters of 1024
                    xq = [xp.tile([128, 1024], DT_MM, tag="xq", name=f"xq{q}_{i}") for i in range(8)]
                    dma_x = nc.sync if DT_MM == F32 else nc.gpsimd
                    for kt in range(8):
                        dma_x.dma_start(
                            xq[kt][:],
                            xT_d[kt * 128 : (kt + 1) * 128, q * 1024 : (q + 1) * 1024],
                        )
                    for ot, dest in enumerate((qT, kTt, vT)):
                        for th in range(2):  # 512-wide halves of the quarter
                            ps = qkps.tile([128, 512], F32, tag="qkps")
                            for kt in range(8):
                                nc.tensor.matmul(
                                    ps[:],
                                    wqkv_g[
                                            :,
                                            kt * 384 + ot * 128 : kt * 384 + (ot + 1) * 128,
                                        ]
                                    ,
                                    xq[kt][:, th * 512 : (th + 1) * 512],
                                    start=(kt == 0),
                                    stop=(kt == 7),
                                )
                            col = q * 1024 + th * 512
                            nc.vector.tensor_copy(dest[:, col : col + 512], ps[:])


            # ---------- Phase 2: attention ----------
            with (
                tc.tile_pool(name="es", bufs=10) as ep,
                tc.tile_pool(name="small", bufs=4) as sp,
                tc.tile_pool(name="unorm", bufs=4) as up,
                tc.tile_pool(name="os", bufs=6) as osp,
                tc.tile_pool(name="s_ps", bufs=4, space="PSUM") as sps,
                tc.tile_pool(name="pv_ps", bufs=2, space="PSUM") as pvps,
            ):
                # V^T [dv, t] -> V [t, dv] via PE transpose at the head of
                # phase 2 (s-tag psum slots) so the PE has dense work across
                # the phase boundary
                for jt in range(32):
                    ptv = sps.tile([128, 128], F32, tag="s", name=f"ptv{jt}")
                    nc.tensor.transpose(ptv[:], vT[:, jt * 128 : (jt + 1) * 128], ident[:])
                    nc.vector.tensor_copy(v1[:, jt * 65 : jt * 65 + 64], ptv[:, 0:64])
                    nc.vector.tensor_copy(v2[:, jt * 65 : jt * 65 + 64], ptv[:, 64:128])

                def emit_po(pb, pib, tt):
                    # out-projection matmuls for an already-normalized block
                    for oh in range(2):
                        tg = pib * 8 + tt
                        po = sps.tile([128, 512], F32, tag="s", name=f"po{pb}_{pib}_{tt}_{oh}")
                        nc.tensor.matmul(
                            po[:],
                            attnT[pb][:, tg * 128 : (tg + 1) * 128],
                            wo_g[:, oh * 512 : (oh + 1) * 512],
                            start=True,
                            stop=True,
                        )
                        ob = osp.tile([128, 512], F32, tag="ob", name=f"ob{pb}_{pib}_{tt}_{oh}")
                        if (tt + oh) % 2 == 0:
                            nc.vector.tensor_copy(ob[:], po[:])
                        else:
                            nc.scalar.copy(ob[:], po[:])
                        row = pb * 2048 + tg * 128
                        nc.sync.dma_start(
                            out_d[row : row + 128, oh * 512 : (oh + 1) * 512], ob[:]
                        )

                prev_block = None
                for b in range(B):
                    for ib in range(2):  # 1024-wide query blocks
                        i0 = b * 2048 + ib * 1024
                        pv = [pvps.tile([65, 1024], F32, tag="pv", name=f"pv{b}_{ib}_{i}") for i in range(2)]
                        for jt in range(16):  # 128-wide key tiles
                            j0 = b * 2048 + jt * 128
                            jv = (b * 16 + jt) * 65
                            # one 1-bank psum tile per (head, i-half): 4 slots ->
                            # deeper S->exp->PV pipeline keeps the PE array dense
                            s_h = [sps.tile([128, 512], F32, tag="s", name=f"s{b}_{ib}_{jt}_{i}") for i in range(4)]
                            e_h = [ep.tile([128, 512], DT_MM, tag="e", name=f"e{b}_{ib}_{jt}_{i}") for i in range(4)]
                            for h in range(2):
                                kTl = kTt[h * 64 : (h + 1) * 64, j0 : j0 + 128]
                                for ih in range(2):
                                    st = s_h[h * 2 + ih]
                                    nc.tensor.matmul(
                                        st[:],
                                        kTl,
                                        qT[
                                            h * 64 : (h + 1) * 64,
                                            i0 + ih * 512 : i0 + (ih + 1) * 512,
                                        ],
                                        start=True,
                                        stop=True,
                                        tile_position=(h * 64, 0),
                                    )
                                    nc.scalar.activation(
                                        e_h[h * 2 + ih][:], st[:], Exp, scale=SCALE
                                    )
                            for h, vv in enumerate((v1, v2)):
                                for ih in range(2):
                                    nc.tensor.matmul(
                                        pv[h][:, ih * 512 : (ih + 1) * 512],
                                        vv[:, jv : jv + 65],
                                        e_h[h * 2 + ih][:],
                                        start=(jt == 0),
                                        stop=(jt == 15),
                                    )
                            if prev_block is not None and jt % 2 == 1:
                                emit_po(prev_block[0], prev_block[1], jt // 2)
                        # --- normalization ---
                        # colsum rows live on psum partition 64; gather each to a
                        # partition-0 [1, 1024] tile (32-aligned reads/writes only)
                        cs_h = [sp.tile([1, 1024], F32, tag="cs", name=f"cs{b}_{ib}_{i}") for i in range(2)]
                        unorm = [up.tile([64, 1024], F32, tag="un", name=f"un{b}_{ib}_{i}") for i in range(2)]
                        for h in range(2):
                            nc.vector.tensor_copy(cs_h[h][:], pv[h][64:65, :])
                            # evacuate unnormalized attnout now so the pv psum
                            # banks free early; normalize later from SBUF
                            nc.vector.tensor_copy(unorm[h][:], pv[h][0:64, :])
                        # transpose 128-wide row chunks onto partitions: col c = blk*2+h
                        pt = pvps.tile([128, 16], F32, tag="pv")
                        for h in range(2):
                            for blk in range(8):
                                nc.tensor.transpose(
                                    pt[:, (blk * 2 + h) : (blk * 2 + h) + 1],
                                    cs_h[h][0:1, blk * 128 : (blk + 1) * 128],
                                    ident[0:1, 0:1],
                                )
                        cst = sp.tile([128, 16], F32, tag="cst")
                        nc.vector.tensor_copy(cst[:], pt[:])
                        rT = sp.tile([128, 16], F32, tag="rT")
                        nc.vector.reciprocal(rT[:], cst[:])
                        # transpose each column back to a [1, 128] row at partition 0
                        r2 = [sp.tile([1, 1024], DT_MM, tag="r2", name=f"r2_{b}_{ib}_{i}") for i in range(2)]
                        for h in range(2):
                            for blk in range(8):
                                c = blk * 2 + h
                                pr1 = pvps.tile([1, 128], F32, tag="pv", name=f"pr{b}_{ib}_{c}")
                                nc.tensor.transpose(pr1[:], rT[:, c : c + 1], ident[:])
                                nc.vector.tensor_copy(
                                    r2[h][0:1, blk * 128 : (blk + 1) * 128], pr1[:]
                                )
                        for h in range(2):
                            rbc = pvps.tile([64, 1024], F32, tag="pv")
                            for ih in range(2):
                                nc.tensor.matmul(
                                    rbc[:, ih * 512 : (ih + 1) * 512],
                                    ones1[:],
                                    r2[h][0:1, ih * 512 : (ih + 1) * 512],
                                    start=True,
                                    stop=True,
                                )
                            rbs = sp.tile([64, 1024], F32, tag="rbs")
                            nc.vector.tensor_copy(rbs[:], rbc[:])
                            nc.vector.tensor_tensor(
                                attnT[b][h * 64 : (h + 1) * 64, ib * 1024 : (ib + 1) * 1024],
                                unorm[h][:],
                                rbs[:],
                                mult,
                            )
                        prev_block = (b, ib)

                # flush the last block's out-projection
                for tt in range(8):
                    emit_po(prev_block[0], prev_block[1], tt)


    nc.compile()
    return nc


_NC = None


def _get_nc():
    global _NC
    if _NC is None:
        _NC = build_nc()
    return _NC


def _gate_pm1(mask):
    """Exact jax fp32 gate: sigmoid(m) > 0.5, encoded as +/-1 for device is_gt(0).

    Computed with the same fp32 logistic rounding as the reference (borderline
    tiny-positive m rounds sigmoid to exactly 0.5 -> gate False, unlike m > 0).
    """
    mask = np.asarray(mask, dtype=np.float32)
    g = (np.float32(1.0) / (np.float32(1.0) + np.exp(-mask))) > np.float32(0.5)
    return np.where(g, np.float32(1.0), np.float32(-1.0))


def make_in_maps(x, qkv_weight, qkv_weight_mask, out_weight, out_weight_mask):
    x = np.asarray(x, dtype=np.float32)
    qkv_weight = np.asarray(qkv_weight, dtype=np.float32)
    qkv_weight_mask = _gate_pm1(qkv_weight_mask)
    out_weight = np.asarray(out_weight, dtype=np.float32)
    out_weight_mask = _gate_pm1(out_weight_mask)

    xT = np.ascontiguousarray(x.reshape(T, DIM).T)
    in_maps = []
    for c in range(NCORES):
        r0 = c * DV  # 2c*64
        sl = slice(r0, r0 + DV)
        w_shard = np.concatenate(
            [qkv_weight[sl], qkv_weight[DIM + r0 : DIM + r0 + DV], qkv_weight[2 * DIM + r0 : 2 * DIM + r0 + DV]],
            axis=0,
        )  # [384, 1024] rows = (q h1,h2 | k h1,h2 | v h1,h2)
        m_shard = np.concatenate(
            [
                qkv_weight_mask[sl],
                qkv_weight_mask[DIM + r0 : DIM + r0 + DV],
                qkv_weight_mask[2 * DIM + r0 : 2 * DIM + r0 + DV],
            ],
            axis=0,
        )
        in_maps.append(
            {
                "xT": xT,
                "wqkvT": np.ascontiguousarray(w_shard.T),
                "mqkvT": np.ascontiguousarray(m_shard.T),
                "woT": np.ascontiguousarray(out_weight[:, sl].T),
                "moT": np.ascontiguousarray(out_weight_mask[:, sl].T),
            }
        )
    return in_maps


LAST_RESULTS = None  # BassKernelResults of the most recent run (for profiling)


def kernel(
    x,
    qkv_weight,
    qkv_weight_mask,
    out_weight,
    out_weight_mask,
    out_bias,
    out_bias_mask,
    _trace=False,
    _tmpdir=None,
):
    global LAST_RESULTS
    from concourse.bass_utils import run_bass_kernel_spmd

    nc = _get_nc()
    in_maps = make_in_maps(x, qkv_weight, qkv_weight_mask, out_weight, out_weight_mask)
    res = run_bass_kernel_spmd(
        nc, in_maps, list(range(NCORES)), trace=_trace, tmpdir=_tmpdir
    )
    LAST_RESULTS = res
    out = np.zeros((T, DIM), dtype=np.float32)
    for r in res.results:
        out += r["out"]
    out_bias = np.asarray(out_bias, dtype=np.float32)
    out_bias_mask = np.asarray(out_bias_mask, dtype=np.float32)
    out += np.where(_gate_pm1(out_bias_mask) > 0.0, out_bias, 0.0)[None, :]
    return out.reshape(B, N, DIM)



# revision 6
# speedup vs baseline: 1.5314x; 1.5314x over previous
"""Trainium2 Bass kernel for nn_Attention_41704132444382.

Masked-linear QKV projection + 16-head attention + masked-linear output
projection. Sharding: batch x head-quad — core c handles batch c//4 and
heads (c%4)*4..(c%4)*4+3. Host sums the 4 per-batch partial outputs and
adds the gated bias.

Per-core layout (all matmul operands bf16; PSUM accumulates fp32):
  - x^T [1024 k, 2048 t] for its batch, on 8 k-partition tiles.
  - Q^T/K^T [64*2, 2048] per head-pair via weight-stationary matmuls.
  - V produced directly as [t, dv] tiles (x tile is the stationary
    operand) — no on-device transposes. A constant ones column at slot 64
    of each [128, 65] V tile makes the PV matmul emit the softmax
    denominator (colsum) on PSUM partition 64 for free.
  - Scores S^T [j keys, i queries]; exp on ScalarE straight from PSUM
    (scale 1/32 folded in; |scores/32| small so no max subtraction),
    one [128, 1024] activation per head-pair, double-buffered.
  - Normalization via PE transpose gather -> DVE reciprocal -> PE
    broadcast, fused into the attnT evacuation.
  - Out-projection accumulates the 4 heads in PSUM (K=128 chains over
    two head-pair attnT tiles); partial [2048, 1024] written fp32.

Emission is software-pipelined: block ib's QK projection chains are
emitted before block ib-1's normalization/out-projection so the PE fills
idle slots of the ScalarE-bound attention steady state.
"""

import sys

import numpy as np

sys.path.insert(0, "/opt/trn_rl_repo")

import concourse.bass as bass
import concourse.mybir as mybir
from concourse import bacc
from concourse.masks import make_identity
from concourse.tile import TileContext

DIM = 1024
HEADS = 16
B = 2
N = 2048  # tokens per batch = tokens per core
NCORES = 8
HPC = 4  # heads per core
DV = HPC * 64  # 256 head-dims per core
SCALE = DIM ** (-0.5)  # 1/32

F32 = mybir.dt.float32
BF16 = mybir.dt.bfloat16

NJT = N // 128  # 16 key tiles
NIB = 4  # query blocks of 512
IBW = 512  # i-block width


def build_nc():
    nc = bacc.Bacc("TRN2", target_bir_lowering=True)
    xT_d = nc.declare_dram_parameter("xT", [DIM, N], BF16, isOutput=False)
    wqkT_d = nc.declare_dram_parameter("wqkT", [DIM, 2 * DV], BF16, isOutput=False)
    wvT_d = nc.declare_dram_parameter("wvT", [DIM, DV], BF16, isOutput=False)
    woT_d = nc.declare_dram_parameter("woT", [DV, DIM], BF16, isOutput=False)
    out_d = nc.declare_dram_parameter("out", [N, DIM], F32, isOutput=True)

    mult = mybir.AluOpType.mult
    Exp = mybir.ActivationFunctionType.Exp

    with TileContext(nc) as tc:
        with tc.tile_pool(name="persist", bufs=1) as pp:
            wqk = pp.tile([128, 8 * 512], BF16)  # [k-part, (kt, qk-col)]
            wv = pp.tile([128, 8 * 256], BF16)  # [k-part, (kt, dv)]
            wo01 = pp.tile([128, 1024], BF16)  # [dv h0|h1, o]
            wo23 = pp.tile([128, 1024], BF16)
            xt = [pp.tile([128, N], BF16, name=f"xt{k}") for k in range(8)]
            qT = [pp.tile([128, N], BF16, name=f"qT{p}") for p in range(2)]  # pair p
            kT = [pp.tile([128, N], BF16, name=f"kT{p}") for p in range(2)]
            v_sb = pp.tile([128, NJT * HPC * 65], BF16)  # [t-part, (jt, h, dv|1)]
            ident = pp.tile([128, 128], F32)
            onesb = pp.tile([1, 64], BF16)

            # ---------- input DMAs ----------
            for k in range(8):
                nc.sync.dma_start(xt[k][:], xT_d[k * 128 : (k + 1) * 128, :])
            nc.sync.dma_start(
                wqk[:].rearrange("p (kt o) -> p kt o", kt=8),
                wqkT_d[:].rearrange("(kt p) o -> p kt o", p=128),
            )
            nc.sync.dma_start(
                wv[:].rearrange("p (kt o) -> p kt o", kt=8),
                wvT_d[:].rearrange("(kt p) o -> p kt o", p=128),
            )
            nc.sync.dma_start(wo01[:], woT_d[0:128, :])
            nc.sync.dma_start(wo23[:], woT_d[128:256, :])

            make_identity(nc, ident[:])
            ones_f = pp.tile([128, 64], F32)
            nc.vector.memset(ones_f[:], 1.0)
            nc.vector.tensor_copy(onesb[:], ones_f[0:1, :])
            # ones column at slot 64 of each 65-wide V block (V writes 0..63)
            nc.vector.tensor_copy(
                v_sb[:].rearrange("p (b c) -> p b c", c=65)[:, :, 64:65],
                ones_f[:, 0 : NJT * HPC].rearrange("p (b c) -> p b c", c=1),
            )

            with (
                tc.tile_pool(name="spool", bufs=2, space="PSUM") as sp,
                tc.tile_pool(name="pvpool", bufs=2, space="PSUM") as pvp,
                tc.tile_pool(name="oppool", bufs=2, space="PSUM") as opp,
                tc.tile_pool(name="epool", bufs=3) as ep,
                tc.tile_pool(name="evac", bufs=2) as vp,
                tc.tile_pool(name="unpool", bufs=8) as up,
                tc.tile_pool(name="obpool", bufs=4) as obp,
            ):
                # ---------- K projection (all t), V direct, Q block 0 ----------
                def emit_qk(o, th):
                    # o: 0/1 -> q pair0/pair1, 2/3 -> k pair0/pair1
                    ps = opp.tile([128, 512], F32, tag="op", name=f"qk{o}_{th}")
                    for kt in range(8):
                        nc.tensor.matmul(
                            ps[:],
                            wqk[:, kt * 512 + o * 128 : kt * 512 + (o + 1) * 128],
                            xt[kt][:, th * 512 : (th + 1) * 512],
                            start=(kt == 0),
                            stop=(kt == 7),
                        )
                    dest = (qT + kT)[o]
                    nc.vector.tensor_copy(dest[:, th * 512 : (th + 1) * 512], ps[:])

                for th in range(4):
                    emit_qk(2, th)
                    emit_qk(3, th)
                for tt in range(16):  # V direct: [128 t, 256 dv]
                    ps = opp.tile([128, 512], F32, tag="op", name=f"vps{tt}")
                    for kt in range(8):
                        nc.tensor.matmul(
                            ps[:, 0:256],
                            xt[kt][:, tt * 128 : (tt + 1) * 128],
                            wv[:, kt * 256 : (kt + 1) * 256],
                            start=(kt == 0),
                            stop=(kt == 7),
                        )
                    nc.vector.tensor_copy(
                        v_sb[:, tt * 4 * 65 : (tt + 1) * 4 * 65].rearrange(
                            "p (h c) -> p h c", c=65
                        )[:, :, 0:64],
                        ps[:, 0:256].rearrange("p (h c) -> p h c", c=64),
                    )
                emit_qk(0, 0)
                emit_qk(1, 0)

                # ---------- attention blocks ----------
                prev = None  # deferred (norm + out-proj) state of prior block

                def emit_norm_outproj(st):
                    ib, cs_sb, unorm = st
                    # gather colsum chunks onto partitions: [1, 2048] -> [128, 16]
                    pt = opp.tile([128, 512], F32, tag="op", name=f"pt{ib}")
                    for i in range(16):  # i = h*4 + c
                        nc.tensor.transpose(
                            pt[:, i : i + 1],
                            cs_sb[0:1, i * 128 : (i + 1) * 128],
                            ident[0:1, 0:1],
                        )
                    rr = vp.tile([128, 16], F32, tag="rr", name=f"rr{ib}")
                    nc.vector.tensor_copy(rr[:], pt[:, 0:16])
                    rcp = vp.tile([128, 16], F32, tag="rcp", name=f"rcp{ib}")
                    nc.vector.reciprocal(rcp[:], rr[:])
                    # transpose reciprocal columns back to partition-0 rows
                    r2 = [
                        vp.tile([1, 512], BF16, tag=f"r2_{h}", name=f"r2_{ib}_{h}")
                        for h in range(4)
                    ]
                    for h in range(4):
                        pr = opp.tile([128, 512], F32, tag="op", name=f"pr{ib}_{h}")
                        for c in range(4):
                            nc.tensor.transpose(
                                pr[0:1, c * 128 : (c + 1) * 128],
                                rcp[:, h * 4 + c : h * 4 + c + 1],
                                ident[:],
                            )
                        nc.vector.tensor_copy(r2[h][0:1, :], pr[0:1, 0:512])
                    attnT = [
                        vp.tile([128, 512], BF16, tag=f"at{p}", name=f"at{ib}_{p}")
                        for p in range(2)
                    ]
                    for h in range(4):
                        rb = opp.tile([128, 512], F32, tag="op", name=f"rb{ib}_{h}")
                        nc.tensor.matmul(
                            rb[0:64, :], onesb[:], r2[h][:], start=True, stop=True
                        )
                        nc.vector.tensor_tensor(
                            attnT[h // 2][(h % 2) * 64 : (h % 2 + 1) * 64, :],
                            unorm[h][:],
                            rb[0:64, :],
                            mult,
                        )
                    # out-projection for this block
                    for tt in range(4):
                        for oh in range(2):
                            po = opp.tile(
                                [128, 512], F32, tag="op", name=f"po{ib}_{tt}_{oh}"
                            )
                            nc.tensor.matmul(
                                po[:],
                                attnT[0][:, tt * 128 : (tt + 1) * 128],
                                wo01[:, oh * 512 : (oh + 1) * 512],
                                start=True,
                                stop=False,
                            )
                            nc.tensor.matmul(
                                po[:],
                                attnT[1][:, tt * 128 : (tt + 1) * 128],
                                wo23[:, oh * 512 : (oh + 1) * 512],
                                start=False,
                                stop=True,
                            )
                            ob = obp.tile(
                                [128, 512], F32, tag="ob", name=f"ob{ib}_{tt}_{oh}"
                            )
                            nc.vector.tensor_copy(ob[:], po[:])
                            nc.sync.dma_start(
                                out_d[
                                    ib * 512 + tt * 128 : ib * 512 + (tt + 1) * 128,
                                    oh * 512 : (oh + 1) * 512,
                                ],
                                ob[:],
                            )

                for ib in range(NIB):
                    i0 = ib * IBW
                    cs_sb = vp.tile([1, 2048], F32, tag="cs", name=f"cs{ib}")
                    unorm = [
                        up.tile([64, 512], F32, tag="un", name=f"un{ib}_{h}")
                        for h in range(4)
                    ]
                    for pair in range(2):
                        pv = [
                            pvp.tile([65, 512], F32, tag="pv", name=f"pv{ib}_{pair}_{hh}")
                            for hh in range(2)
                        ]
                        for jt in range(NJT):
                            s = sp.tile([128, 1024], F32, tag="s", name=f"s{ib}_{pair}_{jt}")
                            for hh in range(2):
                                nc.tensor.matmul(
                                    s[:, hh * 512 : (hh + 1) * 512],
                                    kT[pair][
                                        hh * 64 : (hh + 1) * 64,
                                        jt * 128 : (jt + 1) * 128,
                                    ],
                                    qT[pair][hh * 64 : (hh + 1) * 64, i0 : i0 + IBW],
                                    start=True,
                                    stop=True,
                                )
                            e = ep.tile([128, 1024], BF16, tag="e", name=f"e{ib}_{pair}_{jt}")
                            nc.scalar.activation(e[:], s[:], Exp, scale=SCALE)
                            for hh in range(2):
                                h = pair * 2 + hh
                                jv = (jt * 4 + h) * 65
                                nc.tensor.matmul(
                                    pv[hh][:],
                                    v_sb[:, jv : jv + 65],
                                    e[:, hh * 512 : (hh + 1) * 512],
                                    start=(jt == 0),
                                    stop=(jt == NJT - 1),
                                )
                        for hh in range(2):
                            h = pair * 2 + hh
                            nc.vector.tensor_copy(
                                cs_sb[0:1, h * 512 : (h + 1) * 512], pv[hh][64:65, :]
                            )
                            nc.vector.tensor_copy(unorm[h][:], pv[hh][0:64, :])
                    # prefetch next block's Q projection before this block's
                    # norm/out-proj so its PE work schedules early
                    if ib + 1 < NIB:
                        emit_qk(0, ib + 1)
                        emit_qk(1, ib + 1)
                    if prev is not None:
                        emit_norm_outproj(prev)
                    prev = (ib, cs_sb, unorm)

                emit_norm_outproj(prev)

    nc.compile()
    return nc


_NC = None


def _get_nc():
    global _NC
    if _NC is None:
        _NC = build_nc()
    return _NC


def _gate(mask):
    """Exact jax fp32 gate: sigmoid(m) > 0.5 (fp32 logistic rounding)."""
    mask = np.asarray(mask, dtype=np.float32)
    return (np.float32(1.0) / (np.float32(1.0) + np.exp(-mask))) > np.float32(0.5)


def make_in_maps(x, qkv_weight, qkv_weight_mask, out_weight, out_weight_mask):
    import ml_dtypes

    bf = ml_dtypes.bfloat16
    x = np.asarray(x, dtype=np.float32)
    wq = np.asarray(qkv_weight, dtype=np.float32) * _gate(qkv_weight_mask)
    wo = np.asarray(out_weight, dtype=np.float32) * _gate(out_weight_mask)

    in_maps = []
    for c in range(NCORES):
        b, g = divmod(c, 4)
        r0 = g * DV
        xT = np.ascontiguousarray(x[b].T.astype(bf))
        wqk = np.concatenate(
            [wq[r0 : r0 + DV], wq[DIM + r0 : DIM + r0 + DV]], axis=0
        )  # [512, 1024] rows = (q h0..h3 | k h0..h3)
        in_maps.append(
            {
                "xT": xT,
                "wqkT": np.ascontiguousarray(wqk.T.astype(bf)),
                "wvT": np.ascontiguousarray(wq[2 * DIM + r0 : 2 * DIM + r0 + DV].T.astype(bf)),
                "woT": np.ascontiguousarray(wo[:, r0 : r0 + DV].T.astype(bf)),
            }
        )
    return in_maps


LAST_RESULTS = None  # BassKernelResults of the most recent run (for profiling)


def kernel(
    x,
    qkv_weight,
    qkv_weight_mask,
    out_weight,
    out_weight_mask,
    out_bias,
    out_bias_mask,
    _trace=False,
    _tmpdir=None,
):
    global LAST_RESULTS
    from concourse.bass_utils import run_bass_kernel_spmd

    nc = _get_nc()
    in_maps = make_in_maps(x, qkv_weight, qkv_weight_mask, out_weight, out_weight_mask)
    res = run_bass_kernel_spmd(
        nc, in_maps, list(range(NCORES)), trace=_trace, tmpdir=_tmpdir
    )
    LAST_RESULTS = res
    out = np.zeros((B, N, DIM), dtype=np.float32)
    for c, r in enumerate(res.results):
        out[c // 4] += r["out"]
    out_bias = np.asarray(out_bias, dtype=np.float32)
    out += np.where(_gate(out_bias_mask), out_bias, np.float32(0.0))[None, None, :]
    return out
